# revision 1
# baseline (speedup 1.0000x reference)
"""MoE layer kernel for trn2, expert-parallel across 8 NeuronCores.

Sharding: core c owns routed experts [4c, 4c+4). Router + top-k + combine
weights computed on host (mirroring reference jax ops exactly so routing
decisions match bit-for-bit). Tokens are dispatched per expert on host
(capacity-padded), device computes unweighted SwiGLU expert outputs; the
shared expert is tensor-parallel sharded along the intermediate dim (192
rows per core). Host applies combine weights (scatter-add) and sums the
shared partials.
"""

import os
import sys

import numpy as np

sys.path.insert(0, "/opt/trn_rl_repo")

import jax
import jax.numpy as jnp
import ml_dtypes

import concourse.bass as bass
import concourse.mybir as mybir
import concourse.tile as tile
from concourse import bass_utils
from concourse.masks import make_identity

B, S, H = 1, 512, 2048
T = B * S
I = 1536
E = 32
K = 4
SCALE = 1.8
NCORES = 8
EL = E // NCORES          # local experts per core
IS = I // NCORES          # shared-expert intermediate shard per core
IH = I // 2               # 768, half of intermediate for psum staging
HK = H // 128             # 16 chunks of hidden
IC = I // 128             # 12 chunks of intermediate

BF16 = mybir.dt.bfloat16
F32 = mybir.dt.float32
AF = mybir.ActivationFunctionType


def _build_program(C: int, routed: bool = True, shared: bool = True,
                   repeat: int = 1, hwloop: int = 0,
                   ndev: int = NCORES, compile: bool = True):
    import concourse.bacc as bacc

    nc = bacc.Bacc("TRN2", target_bir_lowering=False, debug=False,
                   num_devices=ndev)

    xg = nc.dram_tensor("xg", [EL, H, C], BF16, kind="ExternalInput").ap()
    wg = nc.dram_tensor("wg", [EL, H, I], BF16, kind="ExternalInput").ap()
    wu = nc.dram_tensor("wu", [EL, H, I], BF16, kind="ExternalInput").ap()
    wd = nc.dram_tensor("wd", [EL, I, H], BF16, kind="ExternalInput").ap()
    sgw = nc.dram_tensor("sgw", [H, IS], BF16, kind="ExternalInput").ap()
    suw = nc.dram_tensor("suw", [H, IS], BF16, kind="ExternalInput").ap()
    sdw = nc.dram_tensor("sdw", [IS, H], BF16, kind="ExternalInput").ap()
    xt = nc.dram_tensor("xt", [H, T], BF16, kind="ExternalInput").ap()
    yr = nc.dram_tensor("yr", [EL, C, H], BF16, kind="ExternalOutput").ap()
    ys = nc.dram_tensor("ys", [T, H], BF16, kind="ExternalOutput").ap()

    if C <= 128:
        widths = [C]
    else:
        assert C % 128 == 0
        widths = [128] * (C // 128)

    with tile.TileContext(nc) as tc:
        with (
            tc.tile_pool(name="const", bufs=1) as const_pool,
            tc.tile_pool(name="xg_sb", bufs=2) as xg_pool,
            tc.tile_pool(name="wgu_sb", bufs=6) as wgu_pool,
            tc.tile_pool(name="wd_sb", bufs=18) as wd_pool,
            tc.tile_pool(name="act_sb", bufs=4) as act_pool,
            tc.tile_pool(name="gut_sb", bufs=2) as gut_pool,
            tc.tile_pool(name="yout_sb", bufs=4) as yout_pool,
            tc.tile_pool(name="gu_ps", bufs=2, space="PSUM") as gu_psum,
            tc.tile_pool(name="tp_ps", bufs=2, space="PSUM") as tp_psum,
            tc.tile_pool(name="y_ps", bufs=2, space="PSUM") as y_psum,
        ):
            ident = const_pool.tile([128, 128], BF16, name="ident")
            make_identity(nc, ident)

            def _emit_body():
                # ---------------- routed experts ----------------
                for e in range(EL if routed else 0):
                    for ct, Cw in enumerate(widths):
                        c0 = ct * 128
                        # gathered tokens for this expert, transposed: [H, Cw]
                        xg_sb = xg_pool.tile([128, HK * Cw], BF16, name="xg_sb")
                        nc.sync.dma_start(
                            out=xg_sb.rearrange("p (k c) -> p k c", c=Cw),
                            in_=xg[e, :, c0:c0 + Cw].rearrange(
                                "(k p) c -> p k c", p=128),
                        )

                        # down-proj weights for this expert: 12 x [128, H]
                        wd_tiles = []
                        for i in range(IC):
                            wd_sb = wd_pool.tile([128, H], BF16, name="wd_sb")
                            nc.sync.dma_start(
                                out=wd_sb, in_=wd[e, bass.ts(i, 128), :])
                            wd_tiles.append(wd_sb)

                        gut_sb = gut_pool.tile([128, IC * Cw], BF16, name="gut_sb")

                        for h in range(2):
                            g_ps = gu_psum.tile([128, IH], F32, name="g_ps", tag="gu")
                            u_ps = gu_psum.tile([128, IH], F32, name="u_ps", tag="gu")
                            for k in range(HK):
                                wg_sb = wgu_pool.tile([128, IH], BF16, name="wg_sb")
                                nc.sync.dma_start(
                                    out=wg_sb,
                                    in_=wg[e, bass.ts(k, 128),
                                           bass.ts(h, IH)])
                                wu_sb = wgu_pool.tile([128, IH], BF16, name="wu_sb")
                                nc.sync.dma_start(
                                    out=wu_sb,
                                    in_=wu[e, bass.ts(k, 128),
                                           bass.ts(h, IH)])
                                lhsT = xg_sb[:, bass.ts(k, Cw)]
                                st = k == 0
                                sp = k == HK - 1
                                nc.tensor.matmul(g_ps[0:Cw, 0:512], lhsT,
                                                 wg_sb[:, 0:512],
                                                 start=st, stop=sp)
                                nc.tensor.matmul(g_ps[0:Cw, 512:IH], lhsT,
                                                 wg_sb[:, 512:IH],
                                                 start=st, stop=sp)
                                nc.tensor.matmul(u_ps[0:Cw, 0:512], lhsT,
                                                 wu_sb[:, 0:512],
                                                 start=st, stop=sp)
                                nc.tensor.matmul(u_ps[0:Cw, 512:IH], lhsT,
                                                 wu_sb[:, 512:IH],
                                                 start=st, stop=sp)

                            silu_sb = act_pool.tile([128, IH], BF16, name="silu_sb")
                            nc.scalar.activation(silu_sb[0:Cw, :], g_ps[0:Cw, :],
                                                 AF.Silu)
                            gu_sb = act_pool.tile([128, IH], BF16, name="gu_sb")
                            nc.vector.tensor_mul(gu_sb[0:Cw, :], silu_sb[0:Cw, :],
                                                 u_ps[0:Cw, :])

                            # transpose gu into [I, tokens] layout for down proj
                            for i in range(IH // 128):
                                tp_ps = tp_psum.tile([128, 128], BF16, name="tp_ps", tag="tp")
                                nc.tensor.transpose(
                                    tp_ps[:, 0:Cw],
                                    gu_sb[0:Cw, bass.ts(i, 128)],
                                    ident[0:Cw, 0:Cw])
                                nc.vector.tensor_copy(
                                    gut_sb[:, bass.ts(h * (IH // 128) + i, Cw)],
                                    tp_ps[:, 0:Cw])

                        # down projection: y[c, H] = sum_i guT_i.T @ wd_i
                        for n in range(H // 512):
                            y_ps = y_psum.tile([128, 512], F32, name="y_ps", tag="y")
                            for i in range(IC):
                                nc.tensor.matmul(
                                    y_ps[0:Cw, :], gut_sb[:, bass.ts(i, Cw)],
                                    wd_tiles[i][:, bass.ts(n, 512)],
                                    start=(i == 0), stop=(i == IC - 1))
                            y_sb = yout_pool.tile([128, 512], BF16, name="y_sb")
                            nc.vector.tensor_copy(y_sb[0:Cw, :], y_ps[0:Cw, :])
                            nc.sync.dma_start(
                                out=yr[e, c0:c0 + Cw, bass.ts(n, 512)],
                                in_=y_sb[0:Cw, :])

                # ---------------- shared expert (I-shard of 192) ----------------
                if shared:
                    # x transposed: [H, T] in sbuf as 16 chunks of [128, T]
                    xt_sb = xg_pool.tile([128, HK * T], BF16, name="xt_sb",
                                         tag="xt", bufs=1)
                    nc.sync.dma_start(
                        out=xt_sb.rearrange("p (k c) -> p k c", c=T),
                        in_=xt.rearrange("(k p) c -> p k c", p=128),
                    )
                    sgw_sb = wgu_pool.tile([128, HK * IS], BF16, name="sgw_sb",
                                           tag="sgw", bufs=1)
                    nc.sync.dma_start(
                        out=sgw_sb.rearrange("p (k c) -> p k c", c=IS),
                        in_=sgw.rearrange("(k p) c -> p k c", p=128),
                    )
                    suw_sb = wgu_pool.tile([128, HK * IS], BF16, name="suw_sb",
                                           tag="suw", bufs=1)
                    nc.sync.dma_start(
                        out=suw_sb.rearrange("p (k c) -> p k c", c=IS),
                        in_=suw.rearrange("(k p) c -> p k c", p=128),
                    )
                    sdw_sb = wd_pool.tile([128, 2 * H], BF16, name="sdw_sb",
                                          tag="sdw", bufs=1)
                    nc.sync.dma_start(
                        out=sdw_sb[:, 0:H], in_=sdw[0:128, :])
                    nc.sync.dma_start(
                        out=sdw_sb[0:IS - 128, H:2 * H], in_=sdw[128:IS, :])

                    # G^T, U^T [IS, T] accumulated over H; M-chunks of 128 + 64
                    mchunks = [(0, 128), (128, IS - 128)]
                    gus_sb = gut_pool.tile([128, 2 * T], BF16, name="gus_sb")
                    for mi, (m0, msz) in enumerate(mchunks):
                        gs_ps = gu_psum.tile([128, T], F32, name="gs_ps", tag="gu")
                        us_ps = tp_psum.tile([128, T], F32, name="us_ps", tag="tp")
                        for k in range(HK):
                            lg = sgw_sb[:, k * IS + m0: k * IS + m0 + msz]
                            lu = suw_sb[:, k * IS + m0: k * IS + m0 + msz]
                            rhs = xt_sb[:, bass.ts(k, T)]
                            st = k == 0
                            sp = k == HK - 1
                            nc.tensor.matmul(gs_ps[0:msz, :], lg, rhs,
                                             start=st, stop=sp)
                            nc.tensor.matmul(us_ps[0:msz, :], lu, rhs,
                                             start=st, stop=sp)
                        ssilu_sb = act_pool.tile([128, T], BF16, name="ssilu_sb")
                        nc.scalar.activation(ssilu_sb[0:msz, :], gs_ps[0:msz, :],
                                             AF.Silu)
                        nc.vector.tensor_mul(
                            gus_sb[0:msz, bass.ts(mi, T)],
                            ssilu_sb[0:msz, :], us_ps[0:msz, :])

                    # down: y_shared[T, H] = (gus)^T.T @ sdw, contract over IS
                    for t in range(T // 128):
                        for n in range(H // 512):
                            y_ps = y_psum.tile([128, 512], F32, name="ys_ps", tag="y")
                            nc.tensor.matmul(
                                y_ps,
                                gus_sb[:, t * 128: t * 128 + 128],
                                sdw_sb[:, n * 512: n * 512 + 512],
                                start=True, stop=False)
                            nc.tensor.matmul(
                                y_ps,
                                gus_sb[0:IS - 128, T + t * 128: T + t * 128 + 128],
                                sdw_sb[0:IS - 128, H + n * 512: H + n * 512 + 512],
                                start=False, stop=True)
                            y_sb = yout_pool.tile([128, 512], BF16, name="ys_sb")
                            nc.vector.tensor_copy(y_sb, y_ps)
                            nc.sync.dma_start(
                                out=ys[bass.ts(t, 128), bass.ts(n, 512)], in_=y_sb)

            if hwloop:
                with tc.For_i(0, hwloop) as _i:
                    _emit_body()
            else:
                for _rep in range(repeat):
                    _emit_body()

    if compile:
        nc.compile()
    return nc


def _prepare(x, router_w, router_b, gate_w, up_w, down_w,
             shared_gate_w, shared_up_w, shared_down_w):
    gate_w = np.asarray(gate_w, np.float32)
    up_w = np.asarray(up_w, np.float32)
    down_w = np.asarray(down_w, np.float32)
    shared_gate_w = np.asarray(shared_gate_w, np.float32)
    shared_up_w = np.asarray(shared_up_w, np.float32)
    shared_down_w = np.asarray(shared_down_w, np.float32)
    xf = np.asarray(x, np.float32).reshape(T, H)

    # --- routing on host, mirroring reference ops exactly ---
    logits = jnp.asarray(xf) @ jnp.asarray(router_w).T + jnp.asarray(router_b)
    top_vals, top_idx = jax.lax.top_k(logits, K)
    rw = jax.nn.softmax(top_vals, axis=-1) * SCALE
    top_idx = np.asarray(top_idx)
    rw = np.asarray(rw, np.float32)

    # per-expert token lists
    tok_of = [np.where((top_idx == e).any(axis=1))[0] for e in range(E)]
    w_of = []
    for e in range(E):
        sel = top_idx[tok_of[e]] == e
        w_of.append((rw[tok_of[e]] * sel).sum(axis=1).astype(np.float32))
    max_n = max(len(t) for t in tok_of)
    if max_n <= 128:
        C = max(32, ((max_n + 31) // 32) * 32)
    else:
        C = ((max_n + 127) // 128) * 128

    bf = ml_dtypes.bfloat16
    xt_np = np.ascontiguousarray(xf.T.astype(bf))          # [H, T]

    in_maps = []
    for c in range(NCORES):
        m = {}
        xg_np = np.zeros((EL, H, C), bf)
        for le in range(EL):
            e = c * EL + le
            idx = tok_of[e]
            if len(idx):
                xg_np[le, :, :len(idx)] = xf[idx].T.astype(bf)
        m["xg"] = xg_np
        es = slice(c * EL, (c + 1) * EL)
        m["wg"] = np.ascontiguousarray(
            np.swapaxes(gate_w[es], 1, 2)).astype(bf)      # [EL, H, I]
        m["wu"] = np.ascontiguousarray(
            np.swapaxes(up_w[es], 1, 2)).astype(bf)
        m["wd"] = np.ascontiguousarray(
            np.swapaxes(down_w[es], 1, 2)).astype(bf)      # [EL, I, H]
        isl = slice(c * IS, (c + 1) * IS)
        m["sgw"] = np.ascontiguousarray(
            shared_gate_w[isl].T).astype(bf)               # [H, IS]
        m["suw"] = np.ascontiguousarray(
            shared_up_w[isl].T).astype(bf)
        m["sdw"] = np.ascontiguousarray(
            shared_down_w[:, isl].T).astype(bf)            # [IS, H]
        m["xt"] = xt_np
        in_maps.append(m)
    return in_maps, tok_of, w_of, C


def kernel(x, router_w, router_b, gate_w, up_w, down_w,
           shared_gate_w, shared_up_w, shared_down_w):
    in_maps, tok_of, w_of, C = _prepare(
        x, router_w, router_b, gate_w, up_w, down_w,
        shared_gate_w, shared_up_w, shared_down_w)

    nc = _build_program(C)
    trace = os.environ.get("MOE_KERNEL_TRACE", "0") == "1"
    res = bass_utils.run_bass_kernel_spmd(
        nc, in_maps, core_ids=list(range(NCORES)), trace=trace)
    kernel.last_results = res

    out = np.zeros((T, H), np.float32)
    for c in range(NCORES):
        out += res.results[c]["ys"]
        for le in range(EL):
            e = c * EL + le
            idx = tok_of[e]
            if len(idx):
                out[idx] += w_of[e][:, None] * \
                    res.results[c]["yr"][le, :len(idx), :]
    return out.reshape(B, S, H).astype(np.float32)



# revision 5
# speedup vs baseline: 2.2314x; 2.2314x over previous
"""MoE layer kernel for trn2, expert-parallel across 8 NeuronCores.

Sharding: core c owns routed experts [4c, 4c+4). Router + top-k + combine
weights computed on host (mirroring reference jax ops exactly so routing
decisions match bit-for-bit). Tokens are dispatched per expert on host
(capacity-padded to C); the shared expert is tensor-parallel sharded along
the intermediate dim (192 rows per core). Host applies combine weights
(scatter-add) and sums the shared partials.

Device-side design (memory-bound problem, ~83 MB/core of weight traffic):
- All weights are pre-arranged on host into [128, N] partition-major blobs
  so every weight matrix loads with 1-4 large (>=1.5 MB) dma_starts at
  near-peak HBM bandwidth (the previous version issued ~350 small DMAs).
- Gate/up matmuls run "transposed" (out[I_chunk, C] = W_chunk^T @ x_chunk,
  contracting H on the partition axis), which produces gu already in the
  [I, tokens] layout the down-projection needs -- no on-chip transposes.
- The per-expert weight stream is split into NS=4 quarters of I so the
  wg/wu/wd tile pools double-buffer 1.57 MB chunks (SBUF-friendly) and DMA
  stays busy; down-proj matmuls are emitted one stage behind gate/up so
  the tensor engine never waits on the silu/mul of the current stage.
- Weight loads split across both HWDGE rings (wg/wu on sync=SP, wd and
  everything else on scalar=ACT) so the rings pipeline independently.
"""

import os
import sys

import numpy as np

sys.path.insert(0, "/opt/trn_rl_repo")

import jax
import jax.numpy as jnp
import ml_dtypes

import concourse.bass as bass
import concourse.mybir as mybir
import concourse.tile as tile
from concourse import bass_utils

B, S, H = 1, 512, 2048
T = B * S
I = 1536
E = 32
K = 4
SCALE = 1.8
NCORES = 8
EL = E // NCORES          # local experts per core
IS = I // NCORES          # shared-expert intermediate shard per core
HK = H // 128             # 16 chunks of hidden (contraction for gate/up)
NS = 4                    # I split into quarters for weight streaming
IQ = I // NS              # 384
JQ = IQ // 128            # 3 chunks of 128 I-rows per quarter
NH = H // 512             # 4 psum-bank-wide chunks of hidden

BF16 = mybir.dt.bfloat16
F32 = mybir.dt.float32
AF = mybir.ActivationFunctionType


def _build_program(C: int, hwloop: int = 0, ndev: int = NCORES,
                   compile: bool = True):
    import concourse.bacc as bacc

    assert C <= 128, "token capacity > 128 not supported by this build"
    nc = bacc.Bacc("TRN2", target_bir_lowering=False, debug=False,
                   num_devices=ndev)

    xg = nc.dram_tensor("xg", [128, EL * HK * C], BF16,
                        kind="ExternalInput").ap()
    wg = nc.dram_tensor("wg", [EL, NS, 128, HK * IQ], BF16,
                        kind="ExternalInput").ap()
    wu = nc.dram_tensor("wu", [EL, NS, 128, HK * IQ], BF16,
                        kind="ExternalInput").ap()
    wd = nc.dram_tensor("wd", [EL, NS, 128, JQ * H], BF16,
                        kind="ExternalInput").ap()
    sgw = nc.dram_tensor("sgw", [128, HK * IS], BF16,
                         kind="ExternalInput").ap()
    suw = nc.dram_tensor("suw", [128, HK * IS], BF16,
                         kind="ExternalInput").ap()
    sdw = nc.dram_tensor("sdw", [128, 2 * H], BF16,
                         kind="ExternalInput").ap()
    xt = nc.dram_tensor("xt", [128, HK * T], BF16,
                        kind="ExternalInput").ap()
    yr = nc.dram_tensor("yr", [EL, C, H], BF16, kind="ExternalOutput").ap()
    ys = nc.dram_tensor("ys", [T, H], BF16, kind="ExternalOutput").ap()

    with tile.TileContext(nc) as tc:
        with (
            tc.tile_pool(name="inp", bufs=1) as inp_pool,
            tc.tile_pool(name="wg_sb", bufs=2) as wg_pool,
            tc.tile_pool(name="wu_sb", bufs=2) as wu_pool,
            tc.tile_pool(name="wd_sb", bufs=2) as wd_pool,
            tc.tile_pool(name="silu", bufs=3) as silu_pool,
            tc.tile_pool(name="gu", bufs=3) as gu_pool,
            tc.tile_pool(name="yout", bufs=2) as yout_pool,
            tc.tile_pool(name="g_ps", bufs=2, space="PSUM") as g_psum,
            tc.tile_pool(name="u_ps", bufs=2, space="PSUM") as u_psum,
            tc.tile_pool(name="y_ps", bufs=1, space="PSUM") as y_psum,
        ):
            def _emit_body():
                # ---- input loads (xg on sync ring first; rest on scalar) --
                xg_sb = inp_pool.tile([128, EL * HK * C], BF16,
                                      name="xg_sb", tag="xg")
                nc.sync.dma_start(out=xg_sb, in_=xg)
                xt_sb = inp_pool.tile([128, HK * T], BF16,
                                      name="xt_sb", tag="xt")
                nc.scalar.dma_start(out=xt_sb, in_=xt)
                sgw_sb = inp_pool.tile([128, HK * IS], BF16,
                                       name="sgw_sb", tag="sgw")
                nc.scalar.dma_start(out=sgw_sb, in_=sgw)
                suw_sb = inp_pool.tile([128, HK * IS], BF16,
                                       name="suw_sb", tag="suw")
                nc.scalar.dma_start(out=suw_sb, in_=suw)
                sdw_sb = inp_pool.tile([128, 2 * H], BF16,
                                       name="sdw_sb", tag="sdw")
                nc.scalar.dma_start(out=sdw_sb, in_=sdw)

                # ---------------- routed experts ----------------
                for e in range(EL):
                    y_ps = y_psum.tile([128, H], F32, name="y_ps")
                    stage = []   # (gu_sb, wd_sb) of the stage awaiting down

                    def _emit_down(s, gu_sb, wd_sb):
                        for j in range(JQ):
                            for n in range(NH):
                                nc.tensor.matmul(
                                    y_ps[0:C, bass.ts(n, 512)],
                                    gu_sb[:, bass.ts(j, C)],
                                    wd_sb[:, j * H + n * 512:
                                          j * H + (n + 1) * 512],
                                    start=(s == 0 and j == 0),
                                    stop=(s == NS - 1 and j == JQ - 1))

                    for s in range(NS):
                        wg_sb = wg_pool.tile([128, HK * IQ], BF16,
                                             name="wg_sb")
                        nc.sync.dma_start(out=wg_sb, in_=wg[e, s])
                        wu_sb = wu_pool.tile([128, HK * IQ], BF16,
                                             name="wu_sb")
                        nc.sync.dma_start(out=wu_sb, in_=wu[e, s])
                        wd_sb = wd_pool.tile([128, JQ * H], BF16,
                                             name="wd_sb")
                        nc.scalar.dma_start(out=wd_sb, in_=wd[e, s])

                        # one PSUM bank per accumulation group (a start=True
                        # clears the has_written bits of the whole bank, so
                        # groups must not share a bank)
                        gu_sb = gu_pool.tile([128, JQ * C], BF16,
                                             name="gu_sb")
                        for j in range(JQ):
                            g_ps = g_psum.tile([128, C], F32, name="g_ps")
                            u_ps = u_psum.tile([128, C], F32, name="u_ps")
                            for k in range(HK):
                                rhs = xg_sb[:, (e * HK + k) * C:
                                            (e * HK + k + 1) * C]
                                lw = wg_sb[:, k * IQ + j * 128:
                                           k * IQ + (j + 1) * 128]
                                nc.tensor.matmul(
                                    g_ps, lw, rhs,
                                    start=(k == 0), stop=(k == HK - 1))
                                lu = wu_sb[:, k * IQ + j * 128:
                                           k * IQ + (j + 1) * 128]
                                nc.tensor.matmul(
                                    u_ps, lu, rhs,
                                    start=(k == 0), stop=(k == HK - 1))
                            s_sb = silu_pool.tile([128, C], BF16,
                                                  name="s_sb")
                            nc.scalar.activation(s_sb, g_ps, AF.Silu)
                            nc.vector.tensor_mul(gu_sb[:, bass.ts(j, C)],
                                                 s_sb, u_ps)

                        # down-proj of the PREVIOUS stage (one-stage skew
                        # hides the silu/mul latency from the tensor queue)
                        if stage:
                            _emit_down(*stage.pop())
                        stage.append((s, gu_sb, wd_sb))

                    _emit_down(*stage.pop())
                    y_sb = yout_pool.tile([128, H], BF16, name="y_sb")
                    nc.vector.tensor_copy(y_sb[0:C, :], y_ps[0:C, :])
                    nc.scalar.dma_start(out=yr[e], in_=y_sb[0:C, :])

                # ---------------- shared expert (I-shard of 192) ----------
                gus_sb = gu_pool.tile([128, 2 * T], BF16, name="gus_sb",
                                      tag="gus", bufs=1)
                for m2, msz in enumerate((128, IS - 128)):
                    gs_ps = g_psum.tile([128, T], F32, name="g_ps")
                    us_ps = u_psum.tile([128, T], F32, name="u_ps")
                    for k in range(HK):
                        rhs = xt_sb[:, bass.ts(k, T)]
                        lg = sgw_sb[:, k * IS + m2 * 128:
                                    k * IS + m2 * 128 + msz]
                        lu = suw_sb[:, k * IS + m2 * 128:
                                    k * IS + m2 * 128 + msz]
                        nc.tensor.matmul(gs_ps[0:msz, :], lg, rhs,
                                         start=(k == 0), stop=(k == HK - 1))
                        nc.tensor.matmul(us_ps[0:msz, :], lu, rhs,
                                         start=(k == 0), stop=(k == HK - 1))
                    ssilu_sb = silu_pool.tile([128, T], BF16,
                                              name="s_sb")
                    nc.scalar.activation(ssilu_sb[0:msz, :], gs_ps[0:msz, :],
                                         AF.Silu)
                    nc.vector.tensor_mul(gus_sb[0:msz, bass.ts(m2, T)],
                                         ssilu_sb[0:msz, :], us_ps[0:msz, :])

                for t in range(T // 128):
                    ys_ps = y_psum.tile([128, H], F32, name="y_ps")
                    for n in range(NH):
                        nc.tensor.matmul(
                            ys_ps[:, bass.ts(n, 512)],
                            gus_sb[:, t * 128:(t + 1) * 128],
                            sdw_sb[:, bass.ts(n, 512)],
                            start=True, stop=False)
                        nc.tensor.matmul(
                            ys_ps[:, bass.ts(n, 512)],
                            gus_sb[0:IS - 128, T + t * 128:T + (t + 1) * 128],
                            sdw_sb[0:IS - 128, H + n * 512:H + (n + 1) * 512],
                            start=False, stop=True)
                    ys_sb = yout_pool.tile([128, H], BF16, name="y_sb")
                    nc.vector.tensor_copy(ys_sb, ys_ps)
                    nc.scalar.dma_start(out=ys[bass.ts(t, 128), :],
                                        in_=ys_sb)

            if hwloop:
                with tc.For_i(0, hwloop) as _i:
                    _emit_body()
            else:
                _emit_body()

    if compile:
        nc.compile()
    return nc


def _prepare(x, router_w, router_b, gate_w, up_w, down_w,
             shared_gate_w, shared_up_w, shared_down_w):
    gate_w = np.asarray(gate_w, np.float32)
    up_w = np.asarray(up_w, np.float32)
    down_w = np.asarray(down_w, np.float32)
    shared_gate_w = np.asarray(shared_gate_w, np.float32)
    shared_up_w = np.asarray(shared_up_w, np.float32)
    shared_down_w = np.asarray(shared_down_w, np.float32)
    xf = np.asarray(x, np.float32).reshape(T, H)

    # --- routing on host, mirroring reference ops exactly ---
    logits = jnp.asarray(xf) @ jnp.asarray(router_w).T + jnp.asarray(router_b)
    top_vals, top_idx = jax.lax.top_k(logits, K)
    rw = jax.nn.softmax(top_vals, axis=-1) * SCALE
    top_idx = np.asarray(top_idx)
    rw = np.asarray(rw, np.float32)

    # per-expert token lists
    tok_of = [np.where((top_idx == e).any(axis=1))[0] for e in range(E)]
    w_of = []
    for e in range(E):
        sel = top_idx[tok_of[e]] == e
        w_of.append((rw[tok_of[e]] * sel).sum(axis=1).astype(np.float32))
    max_n = max(len(t) for t in tok_of)
    C = max(32, ((max_n + 31) // 32) * 32)
    assert C <= 128

    bf = ml_dtypes.bfloat16
    xfb = xf.astype(bf)
    # xt: [128, HK*T], row p block k = x[:, k*128+p]
    xt_np = np.ascontiguousarray(
        xfb.T.reshape(HK, 128, T).transpose(1, 0, 2)).reshape(128, HK * T)

    in_maps = []
    for c in range(NCORES):
        m = {}
        es = slice(c * EL, (c + 1) * EL)

        # xg: [128, EL*HK*C], row p block (e, k) = gathered tokens
        xg_np = np.zeros((128, EL, HK, C), bf)
        for le in range(EL):
            idx = tok_of[c * EL + le]
            if len(idx):
                xg_np[:, le, :, :len(idx)] = (
                    xfb[idx].T.reshape(HK, 128, len(idx)).transpose(1, 0, 2))
        m["xg"] = xg_np.reshape(128, EL * HK * C)

        # wg/wu: [EL, NS, 128, HK*IQ]; elem [e,s,p,k*IQ+i] = W[e, s*IQ+i, k*128+p]
        def _gu_layout(w):
            a = w[es].reshape(EL, NS, IQ, HK, 128)
            return np.ascontiguousarray(
                a.transpose(0, 1, 4, 3, 2)).astype(bf).reshape(
                    EL, NS, 128, HK * IQ)
        m["wg"] = _gu_layout(gate_w)
        m["wu"] = _gu_layout(up_w)

        # wd: [EL, NS, 128, JQ*H]; elem [e,s,p,j*H+h] = down[e, h, s*IQ+j*128+p]
        a = down_w[es].reshape(EL, H, NS, JQ, 128)
        m["wd"] = np.ascontiguousarray(
            a.transpose(0, 2, 4, 3, 1)).astype(bf).reshape(
                EL, NS, 128, JQ * H)

        # shared gate/up: [128, HK*IS]; elem [p, k*IS+mm] = W[c*IS+mm, k*128+p]
        isl = slice(c * IS, (c + 1) * IS)

        def _s_layout(w):
            return np.ascontiguousarray(
                w[isl].T.reshape(HK, 128, IS).transpose(1, 0, 2)).astype(
                    bf).reshape(128, HK * IS)
        m["sgw"] = _s_layout(shared_gate_w)
        m["suw"] = _s_layout(shared_up_w)

        # shared down: [128, 2*H]; block m2 row p = down_shared[:, c*IS+m2*128+p]
        sd = shared_down_w[:, isl].T.astype(bf)     # [IS, H]
        sdw_np = np.zeros((128, 2, H), bf)
        sdw_np[:, 0, :] = sd[0:128]
        sdw_np[0:IS - 128, 1, :] = sd[128:IS]
        m["sdw"] = sdw_np.reshape(128, 2 * H)

        m["xt"] = xt_np
        in_maps.append(m)
    return in_maps, tok_of, w_of, C


def kernel(x, router_w, router_b, gate_w, up_w, down_w,
           shared_gate_w, shared_up_w, shared_down_w):
    in_maps, tok_of, w_of, C = _prepare(
        x, router_w, router_b, gate_w, up_w, down_w,
        shared_gate_w, shared_up_w, shared_down_w)

    nc = _build_program(C)
    trace = os.environ.get("MOE_KERNEL_TRACE", "0") == "1"
    res = bass_utils.run_bass_kernel_spmd(
        nc, in_maps, core_ids=list(range(NCORES)), trace=trace)
    kernel.last_results = res

    out = np.zeros((T, H), np.float32)
    for c in range(NCORES):
        out += res.results[c]["ys"]
        for le in range(EL):
            e = c * EL + le
            idx = tok_of[e]
            if len(idx):
                out[idx] += w_of[e][:, None] * \
                    res.results[c]["yr"][le, :len(idx), :]
    return out.reshape(B, S, H).astype(np.float32)
